# revision 5
# baseline (speedup 1.0000x reference)
"""Trainium2 Bass kernel for nn_Decoder (scatter + gaussian conv + CTF filter).

Self-contained: hardcodes shapes/sharding for
  alignment (16,6), shifts (16,2), coords (500000,3), values (500000,),
  ctf (16,256,129) -> out (16,256,256) float32, 8 NeuronCores.

Sharding: pure data-parallel over the batch; each core handles 2 images.
Inside each core:
  - scatter: for each 128-point chunk, build the two 2-sparse bilinear
    profile matrices (y-profile and value-weighted x-profile) as fp16
    one-hot rows via the GPSIMD local_scatter ucode op, then accumulate
    the 256x256 image in PSUM with PE matmuls yprof^T @ xw.
  - conv+FFT+CTF+iFFT: gaussian conv is folded into precomputed DFT
    matrices; the whole linear chain is fp32 matmuls + PE transposes.
"""
import os
import sys
if '/opt/trn_rl_repo' not in sys.path:
    sys.path.insert(0, '/opt/trn_rl_repo')

import numpy as np
import concourse.bass as bass
import concourse.bacc as bacc
import concourse.mybir as mybir
from concourse.tile import TileContext
from concourse.bass_utils import run_bass_kernel_spmd

F16 = mybir.dt.float16
F32 = mybir.dt.float32
I16 = mybir.dt.int16
I32 = mybir.dt.int32
OP = mybir.AluOpType

XSIZE = 256
KX = 129
N_PTS = 500000
B_FULL = 16
N_CORES = 8
IMGS = 2                    # images per core
NCHUNK = 3920               # point chunks per image (128 pts each), padded
NPAD = NCHUNK * 128         # 501760 padded points
G = 7                       # chunks per local_scatter call
NE = 256 * G                # 1792 dst columns per call
BODY_C = 28                 # chunks per For_i body (= 4 lscat groups)
N_ITER = NCHUNK // BODY_C   # 140
PH_C = 14                   # chunks per phase (2 phases per body)


# ---------------------------------------------------------------- host mats
def _build_mats():
    n = XSIZE
    y = np.arange(n)
    ax = np.arange(5, dtype=np.float64) - 2.0
    g = np.exp(-(ax ** 2) / 2.0)
    gn = g / g.sum()
    Gm = np.zeros((n, n))
    for d in range(-2, 3):
        idx = np.arange(max(0, -d), min(n, n - d))
        Gm[idx, idx + d] = gn[d + 2]
    F = np.exp(-2j * np.pi * np.outer(y, y) / n)
    A = F @ Gm                                               # (256,256)
    Bh = np.exp(-2j * np.pi * np.outer(np.arange(KX), y) / n) @ Gm
    Bm = np.zeros((n, n), complex)
    Bm[:KX] = Bh                                             # kx zero-padded
    IFy = np.exp(+2j * np.pi * np.outer(y, y) / n) / n
    c = np.ones(KX)
    c[1:-1] = 2.0
    EXh = (np.exp(+2j * np.pi * np.outer(y, np.arange(KX)) / n) * c[None, :]) / n
    EX = np.zeros((n, n), complex)
    EX[:, :KX] = EXh

    def lhsT(M):  # (256,256) -> transposed, chunked (2,128,256) f32
        t = np.ascontiguousarray(M.T.reshape(2, 128, 256))
        return t.astype(np.float32)

    mats = {
        "ATr": lhsT(A.real), "ATi": lhsT(A.imag),
        "BrT": lhsT(Bm.real), "BiT": lhsT(Bm.imag), "nBiT": lhsT(-Bm.real * 0 - Bm.imag),
        "IFrT": lhsT(IFy.real), "IFiT": lhsT(IFy.imag), "nIFiT": lhsT(-IFy.imag),
        "EXrT": lhsT(EX.real), "nEXiT": lhsT(-EX.imag),
        "ident": np.eye(128, dtype=np.float32),
    }
    return mats


MAT_NAMES = ["ATr", "ATi", "BrT", "BiT", "nBiT", "IFrT", "IFiT", "nIFiT",
             "EXrT", "nEXiT"]


# ---------------------------------------------------------------- bass build
def _build_nc():
    nc = bacc.Bacc()
    xt_in = nc.declare_dram_parameter("xt", [128, NCHUNK], F32, isOutput=False)
    yt_in = nc.declare_dram_parameter("yt", [128, NCHUNK], F32, isOutput=False)
    zt_in = nc.declare_dram_parameter("zt", [128, NCHUNK], F32, isOutput=False)
    vt_in = nc.declare_dram_parameter("vt", [128, NCHUNK], F32, isOutput=False)
    sc_in = nc.declare_dram_parameter("sc", [IMGS, 8], F32, isOutput=False)
    xoff_in = nc.declare_dram_parameter("xoff", [1, BODY_C], F32, isOutput=False)
    ctf_in = nc.declare_dram_parameter("ctfT", [IMGS, 256, 256], F32, isOutput=False)
    mat_in = {m: nc.declare_dram_parameter(m, [2, 128, 256], F32, isOutput=False)
              for m in MAT_NAMES}
    id_in = nc.declare_dram_parameter("ident", [128, 128], F32, isOutput=False)
    out_d = nc.declare_dram_parameter("out", [IMGS, 256, 256], F32, isOutput=True)

    with TileContext(nc) as tc:
        with tc.tile_pool(name="inp", bufs=1) as inp, \
             tc.tile_pool(name="mat", bufs=1) as matp, \
             tc.tile_pool(name="prep", bufs=2) as prep, \
             tc.tile_pool(name="dstp", bufs=1) as dstp, \
             tc.tile_pool(name="work", bufs=1) as work, \
             tc.tile_pool(name="accp", bufs=1, space="PSUM") as accp, \
             tc.tile_pool(name="eps", bufs=4, space="PSUM") as eps:

            # ---------------- load inputs ----------------
            xt = inp.tile([128, NCHUNK], F32)
            yt = inp.tile([128, NCHUNK], F32)
            zt = inp.tile([128, NCHUNK], F32)
            vt = inp.tile([128, NCHUNK], F32)
            nc.sync.dma_start(xt[:], xt_in[:])
            nc.sync.dma_start(yt[:], yt_in[:])
            nc.sync.dma_start(zt[:], zt_in[:])
            nc.sync.dma_start(vt[:], vt_in[:])

            sc1 = [inp.tile([1, 8], F32, name=f'sc1_{i}') for i in range(IMGS)]
            sc128 = [inp.tile([128, 8], F32, name=f'sc128_{i}') for i in range(IMGS)]
            for b in range(IMGS):
                nc.sync.dma_start(sc1[b][:], sc_in[b:b + 1, :])
                nc.gpsimd.partition_broadcast(sc128[b][:], sc1[b][:])
            xoff1 = inp.tile([1, BODY_C], F32)
            xoff = inp.tile([128, BODY_C], F32)
            nc.sync.dma_start(xoff1[:], xoff_in[:])
            nc.gpsimd.partition_broadcast(xoff[:], xoff1[:])

            mats = {}
            for m in MAT_NAMES:
                t0 = matp.tile([128, 256], F32, tag=f"{m}0")
                t1 = matp.tile([128, 256], F32, tag=f"{m}1")
                nc.sync.dma_start(t0[:], mat_in[m][0])
                nc.sync.dma_start(t1[:], mat_in[m][1])
                mats[m] = (t0, t1)
            ident = matp.tile([128, 128], F32)
            nc.sync.dma_start(ident[:], id_in[:])
            ctfs = []
            for b in range(IMGS):
                c0 = matp.tile([128, 256], F32, tag=f"ctf{b}0")
                c1 = matp.tile([128, 256], F32, tag=f"ctf{b}1")
                nc.sync.dma_start(c0[:], ctf_in[b, 0:128, :])
                nc.sync.dma_start(c1[:], ctf_in[b, 128:256, :])
                ctfs.append((c0, c1))

            zero16 = inp.tile([128, 256], F16)
            nc.vector.memset(zero16[:], 0.0)

            # ---------------- PSUM accumulators ----------------
            acc = [[accp.tile([128, 256], F32, tag=f"acc{b}{h}",
                               name=f"acc_{b}_{h}")
                    for h in range(2)] for b in range(IMGS)]
            for b in range(IMGS):
                for h in range(2):
                    nc.tensor.matmul(acc[b][h][:], zero16[:, 0:128],
                                     zero16[:], start=True, stop=False)

            # ---------------- main scatter loop ----------------
            def prep_side(b, base, coord_t, is_x, idx_t, dat_t):
                """Emit DVE prep for one (image, phase, axis).

                coord_t: xt or yt; writes idx_t (128,PH_C,2) i16 and
                dat_t (128,PH_C,2) f16.
                """
                sc = sc128[b]
                k0 = 0 if is_x else 3
                cstc = 6 if is_x else 7
                t0 = prep.tile([128, PH_C], F32, tag="p_t0")
                nc.vector.tensor_scalar(
                    t0[:], xt[:, bass.DynSlice(base, PH_C)],
                    sc[:, k0:k0 + 1], sc[:, cstc:cstc + 1],
                    op0=OP.mult, op1=OP.add)
                t1 = prep.tile([128, PH_C], F32, tag="p_t1")
                nc.vector.scalar_tensor_tensor(
                    t1[:], yt[:, bass.DynSlice(base, PH_C)],
                    sc[:, k0 + 1:k0 + 2], t0[:], op0=OP.mult, op1=OP.add)
                t2 = prep.tile([128, PH_C], F32, tag="p_t2")
                nc.vector.scalar_tensor_tensor(
                    t2[:], zt[:, bass.DynSlice(base, PH_C)],
                    sc[:, k0 + 2:k0 + 3], t1[:], op0=OP.mult, op1=OP.add)
                return t2

            with tc.For_i(0, N_ITER, 1) as it:
                for ph in range(2):
                    base = it * BODY_C + ph * PH_C
                    dsts = {}
                    for b in range(IMGS):
                        for is_x in (True, False):
                            ax_n = "x" if is_x else "y"
                            co = prep_side(b, base, xt if is_x else yt,
                                           is_x, None, None)
                            # add per-chunk 256*slot offset
                            cxo = prep.tile([128, PH_C], F32, tag="p_cxo")
                            nc.vector.tensor_tensor(
                                cxo[:], co[:],
                                xoff[:, ph * PH_C:(ph + 1) * PH_C], op=OP.add)
                            # floor
                            ii = prep.tile([128, PH_C], I32, tag="p_ii")
                            nc.vector.tensor_copy(ii[:], cxo[:])
                            dd = prep.tile([128, PH_C], F32, tag="p_dd")
                            nc.vector.tensor_copy(dd[:], ii[:])
                            gt = prep.tile([128, PH_C], F32, tag="p_gt")
                            nc.vector.tensor_tensor(gt[:], dd[:], cxo[:],
                                                    op=OP.is_gt)
                            i0f = prep.tile([128, PH_C], F32, tag="p_i0f")
                            nc.vector.tensor_tensor(i0f[:], dd[:], gt[:],
                                                    op=OP.subtract)
                            fr = prep.tile([128, PH_C], F32, tag="p_fr")
                            nc.vector.tensor_tensor(fr[:], cxo[:], i0f[:],
                                                    op=OP.subtract)
                            idx_t = prep.tile([128, PH_C, 2], I16,
                                              tag=f"idx{b}{ax_n}")
                            nc.vector.tensor_copy(idx_t[:, :, 0], i0f[:])
                            nc.vector.tensor_scalar(
                                idx_t[:, :, 1], i0f[:], 1.0, None, op0=OP.add)
                            dat_t = prep.tile([128, PH_C, 2], F16,
                                              tag=f"dat{b}{ax_n}")
                            if is_x:
                                vfx = prep.tile([128, PH_C], F32, tag="p_vfx")
                                nc.vector.tensor_tensor(
                                    vfx[:], vt[:, bass.DynSlice(base, PH_C)],
                                    fr[:], op=OP.mult)
                                nc.vector.tensor_tensor(
                                    dat_t[:, :, 0],
                                    vt[:, bass.DynSlice(base, PH_C)],
                                    vfx[:], op=OP.subtract)
                                nc.vector.tensor_copy(dat_t[:, :, 1], vfx[:])
                            else:
                                nc.vector.tensor_scalar(
                                    dat_t[:, :, 0], fr[:], -1.0, 1.0,
                                    op0=OP.mult, op1=OP.add)
                                nc.vector.tensor_copy(dat_t[:, :, 1], fr[:])
                            # two local_scatter calls (7 chunks each)
                            for k in range(2):
                                dt = dstp.tile([128, NE], F16,
                                               tag=f"dst{b}{ax_n}{k}")
                                nc.gpsimd.local_scatter(
                                    dt[:],
                                    dat_t[:, 7 * k:7 * (k + 1), :],
                                    idx_t[:, 7 * k:7 * (k + 1), :],
                                    channels=128, num_elems=NE, num_idxs=2 * G)
                                dsts[(b, ax_n, k)] = dt
                    # matmuls for this phase
                    for b in range(IMGS):
                        for k in range(2):
                            yd = dsts[(b, "y", k)]
                            xd = dsts[(b, "x", k)]
                            for s in range(G):
                                rhs = xd[:, 256 * s:256 * (s + 1)]
                                for h in range(2):
                                    lhsT = yd[:, 256 * s + 128 * h:
                                              256 * s + 128 * (h + 1)]
                                    nc.tensor.matmul(acc[b][h][:], lhsT, rhs,
                                                     start=False, stop=False)

            for b in range(IMGS):
                for h in range(2):
                    nc.tensor.matmul(acc[b][h][:], zero16[:, 0:128],
                                     zero16[:], start=False, stop=True)

            # ---------------- epilogue: conv+FFT+CTF+iFFT ----------------
            def mm_pair(out_ps, lT, rhs_tiles, extra=None, first=True):
                """out_ps += sum_kc lT[kc]^T @ rhs_tiles[kc] (+ extra pair)."""
                ops = []
                for kc in range(2):
                    ops.append((lT[kc], rhs_tiles[kc]))
                if extra is not None:
                    lT2, rhs2 = extra
                    for kc in range(2):
                        ops.append((lT2[kc], rhs2[kc]))
                for j, (lt, rh) in enumerate(ops):
                    nc.tensor.matmul(out_ps[:], lt, rh,
                                     start=(first and j == 0),
                                     stop=(j == len(ops) - 1))

            def transpose_mat(src_tiles, tag):
                """src: 2 SBUF tiles (128,256) = (256,256) matrix -> transposed."""
                dst = [work.tile([128, 256], F32, tag=f"{tag}{m}",
                                 name=f"tr_{tag}_{m}")
                       for m in range(2)]
                for a in range(2):
                    for bcol in range(2):
                        pt = eps.tile([128, 128], F32, tag="ep")
                        nc.tensor.transpose(
                            pt[:], src_tiles[a][:, 128 * bcol:128 * (bcol + 1)],
                            ident[:])
                        nc.vector.tensor_copy(
                            dst[bcol][:, 128 * a:128 * (a + 1)], pt[:])
                return dst

            def cmul_stage(lr, li, nli, rhs_r, rhs_i, tag):
                """Complex matmul stage: returns (out_r, out_i) SBUF tiles.

                out_r = lr^T@rhs_r + nli^T@rhs_i ; out_i = lr^T@rhs_i + li^T@rhs_r
                Each output is 2 M-half tiles (128,256).
                """
                outr, outi = [], []
                for m in range(2):
                    lrm = [lr[kc][:, 128 * m:128 * (m + 1)] for kc in range(2)]
                    lim = [li[kc][:, 128 * m:128 * (m + 1)] for kc in range(2)]
                    nlim = [nli[kc][:, 128 * m:128 * (m + 1)] for kc in range(2)]
                    pr = eps.tile([128, 256], F32, tag="ep")
                    mm_pair(pr, lrm, rhs_r, extra=(nlim, rhs_i))
                    tr = work.tile([128, 256], F32, tag=f"{tag}r{m}")
                    nc.vector.tensor_copy(tr[:], pr[:])
                    outr.append(tr)
                    pi = eps.tile([128, 256], F32, tag="ep")
                    mm_pair(pi, lrm, rhs_i, extra=(lim, rhs_r))
                    ti = work.tile([128, 256], F32, tag=f"{tag}i{m}")
                    nc.vector.tensor_copy(ti[:], pi[:])
                    outi.append(ti)
                return outr, outi

            for b in range(IMGS):
                img_sb = [work.tile([128, 256], F32, tag=f"img{h}",
                                    name=f"img_sb_{h}")
                          for h in range(2)]
                for h in range(2):
                    nc.vector.tensor_copy(img_sb[h][:], acc[b][h][:])
                # U = A @ img
                Ur, Ui = [], []
                for m in range(2):
                    for part, lst in (("r", Ur), ("i", Ui)):
                        mat = mats["ATr" if part == "r" else "ATi"]
                        ps = eps.tile([128, 256], F32, tag="ep")
                        mm_pair(ps, [mat[kc][:, 128 * m:128 * (m + 1)]
                                     for kc in range(2)], img_sb)
                        t = work.tile([128, 256], F32, tag=f"U{part}{m}")
                        nc.vector.tensor_copy(t[:], ps[:])
                        lst.append(t)
                UTr = transpose_mat(Ur, "UTr")
                UTi = transpose_mat(Ui, "UTi")
                # ST = B @ UT ; then ctf
                STr, STi = cmul_stage(mats["BrT"], mats["BiT"], mats["nBiT"],
                                      UTr, UTi, "ST")
                Spr, Spi = [], []
                for m in range(2):
                    tr = work.tile([128, 256], F32, tag=f"Spr{m}")
                    nc.vector.tensor_tensor(tr[:], STr[m][:], ctfs[b][m][:],
                                            op=OP.mult)
                    Spr.append(tr)
                    ti = work.tile([128, 256], F32, tag=f"Spi{m}")
                    nc.vector.tensor_tensor(ti[:], STi[m][:], ctfs[b][m][:],
                                            op=OP.mult)
                    Spi.append(ti)
                SpTr = transpose_mat(Spr, "SpTr")
                SpTi = transpose_mat(Spi, "SpTi")
                # W = IFy @ Sp
                Wr, Wi = cmul_stage(mats["IFrT"], mats["IFiT"], mats["nIFiT"],
                                    SpTr, SpTi, "W")
                WTr = transpose_mat(Wr, "WTr")
                WTi = transpose_mat(Wi, "WTi")
                # outT = Re(EX @ WT)
                for m in range(2):
                    po = eps.tile([128, 256], F32, tag="ep")
                    mm_pair(po, [mats["EXrT"][kc][:, 128 * m:128 * (m + 1)]
                                 for kc in range(2)], WTr,
                            extra=([mats["nEXiT"][kc][:, 128 * m:128 * (m + 1)]
                                    for kc in range(2)], WTi))
                    ot = work.tile([128, 256], F32, tag=f"outT{m}")
                    nc.vector.tensor_copy(ot[:], po[:])
                    nc.sync.dma_start(out_d[b, 128 * m:128 * (m + 1), :], ot[:])
    nc.finalize()
    return nc


_NC_CACHE = None


def _get_nc():
    global _NC_CACHE
    if _NC_CACHE is None:
        _NC_CACHE = _build_nc()
    return _NC_CACHE


# ---------------------------------------------------------------- host entry
def build_in_maps(alignment, shifts, coords, values, ctf):
    alignment = np.asarray(alignment, np.float32)
    shifts = np.asarray(shifts, np.float32)
    coords = np.asarray(coords, np.float32)
    values = np.asarray(values, np.float32)
    ctf = np.asarray(ctf, np.float32)

    # pad points; pad coords with a copy of point 0 (in range), v=0
    cpad = np.empty((NPAD, 3), np.float32)
    cpad[:N_PTS] = coords
    cpad[N_PTS:] = coords[0]
    vpad = np.zeros((NPAD,), np.float32)
    vpad[:N_PTS] = values
    fx = np.ascontiguousarray(cpad[:, 0].reshape(128, NCHUNK))
    fy = np.ascontiguousarray(cpad[:, 1].reshape(128, NCHUNK))
    fz = np.ascontiguousarray(cpad[:, 2].reshape(128, NCHUNK))
    fv = np.ascontiguousarray(vpad.reshape(128, NCHUNK))

    xoffrow = (256.0 * (np.arange(BODY_C) % G)).astype(np.float32)[None, :]
    mats = _build_mats()

    in_maps = []
    for c in range(N_CORES):
        b0 = IMGS * c
        sc = np.zeros((IMGS, 8), np.float32)
        for b in range(IMGS):
            sc[b, 0:6] = alignment[b0 + b]
            sc[b, 6] = 128.0 - shifts[b0 + b, 0]
            sc[b, 7] = 128.0 - shifts[b0 + b, 1]
        ctfT = np.zeros((IMGS, 256, 256), np.float32)
        ctfT[:, :KX, :] = np.transpose(ctf[b0:b0 + IMGS], (0, 2, 1))
        m = {"xt": fx, "yt": fy, "zt": fz, "vt": fv,
             "sc": sc, "xoff": xoffrow, "ctfT": ctfT,
             "ident": mats["ident"]}
        for name in MAT_NAMES:
            m[name] = mats[name]
        in_maps.append(m)
    return in_maps


def unshard_output(results):
    out = np.empty((B_FULL, 256, 256), np.float32)
    for c in range(N_CORES):
        o = results[c]["out"]              # (2, 256, 256) x-major
        for b in range(IMGS):
            out[IMGS * c + b] = o[b].T
    return out


def kernel(alignment, shifts, coords, values, ctf):
    in_maps = build_in_maps(alignment, shifts, coords, values, ctf)
    nc = _get_nc()
    global LAST_RES
    res = run_bass_kernel_spmd(nc, in_maps, list(range(N_CORES)),
                               tmpdir=os.environ.get("BASS_TMPDIR"))
    LAST_RES = res
    return unshard_output(res.results)


if __name__ == "__main__":
    d = np.load("/root/problem/work/ref_cache.npz")
    ins = {k: d[k] for k in ["alignment", "shifts", "coords", "values", "ctf"]}
    o = kernel(**ins)
    ref = d["ref"]
    err = np.abs(o - ref).max() / np.abs(ref).max()
    print("rel err:", err)

